# revision 5
# baseline (speedup 1.0000x reference)
"""Trainium2 Bass kernel for a dense transformer block (RMSNorm -> causal MHA
-> residual -> RMSNorm -> GLU FFN -> residual), SPMD across 8 NeuronCores.

Sharding: tensor-parallel attention (2 heads/core) -> AllToAll of per-head
attention outputs -> token-parallel proj + FFN (512 tokens/core).

All matmuls run in bf16 with fp32 PSUM accumulation; norms/softmax math in fp32.
"""
import numpy as np
import ml_dtypes

import concourse.bass as bass
import concourse.tile as tile
from concourse import bacc, mybir
from concourse.bass_utils import run_bass_kernel_spmd

F32 = mybir.dt.float32
BF16 = mybir.dt.bfloat16
AF = mybir.ActivationFunctionType
ALU = mybir.AluOpType

B, T, D, H, HD = 2, 2048, 1024, 16, 64
NCORES = 8
HPC = H // NCORES          # heads per core = 2
E2 = HPC * HD              # per-core attention channels = 128
NTOK = B * T               # 4096
TS = NTOK // NCORES        # tokens per core slice = 512
EPS = 1e-6
NDC = D // 128             # 8 D-chunks
CORE_IDS = list(range(NCORES))

_CACHE = {}
_O_STASH = {}


def _build():
    nc = bacc.Bacc("TRN2", target_bir_lowering=False, debug=False,
                   num_devices=NCORES)

    # ---- DRAM I/O ----
    tensors = dict(
        xb=nc.dram_tensor("xb", [NTOK, D], BF16, kind="ExternalInput"),
        wq=nc.dram_tensor("wq", [D, E2], BF16, kind="ExternalInput"),
        wk=nc.dram_tensor("wk", [D, E2], BF16, kind="ExternalInput"),
        wv=nc.dram_tensor("wv", [D, E2], BF16, kind="ExternalInput"),
        wp=nc.dram_tensor("wp", [D, D], BF16, kind="ExternalInput"),
        w1=nc.dram_tensor("w1", [D, 8 * D], BF16, kind="ExternalInput"),
        b1=nc.dram_tensor("b1", [8 * D], F32, kind="ExternalInput"),
        w2=nc.dram_tensor("w2", [4 * D, D], BF16, kind="ExternalInput"),
        b2=nc.dram_tensor("b2", [1, D], BF16, kind="ExternalInput"),
        xsp=nc.dram_tensor("xsp", [TS, D], F32, kind="ExternalInput"),
        mk=nc.dram_tensor("mk", [128, 896], BF16, kind="ExternalInput"),
        sel=nc.dram_tensor("sel", [8, 8, 64], BF16, kind="ExternalInput"),
        out=nc.dram_tensor("out", [TS, D], F32, kind="ExternalOutput"),
    )

    with tile.TileContext(nc) as tc:
        _body(nc, tc, tensors)
    nc.compile()
    return nc


def _body(nc, tc, tn):
    ts = bass.ts
    xb_d, wq_d, wk_d, wv_d, wp_d = tn["xb"], tn["wq"], tn["wk"], tn["wv"], tn["wp"]
    w1_d, b1_d, w2_d, b2_d = tn["w1"], tn["b1"], tn["w2"], tn["b2"]
    xsp_d, mk_d, out_d = tn["xsp"], tn["mk"], tn["out"]
    sel_d = tn["sel"]

    persist = tc.tile_pool(name="persist", bufs=1)
    pp = persist.__enter__()
    dram = tc.tile_pool(name="dram", bufs=1, space="DRAM")
    dd = dram.__enter__()

    # persistent small tensors
    eps_t = pp.tile([128, 1], F32)
    nc.vector.memset(eps_t, EPS)
    ones1 = pp.tile([1, 128], BF16)
    nc.vector.memset(ones1, 1.0)
    sel8 = pp.tile([8, 8, HD], BF16)      # sel8[k, r, :] = (k == r)
    nc.sync.dma_start(out=sel8, in_=sel_d.ap())
    b2_sb = pp.tile([1, D], BF16)
    nc.sync.dma_start(out=b2_sb, in_=b2_d.ap())
    b1a_sb = pp.tile([128, 32], F32)
    nc.sync.dma_start(out=b1a_sb,
                      in_=b1_d.ap()[:4 * D].rearrange("(i p) -> p i", p=128))
    b1b_sb = pp.tile([128, 32], F32)
    nc.sync.dma_start(out=b1b_sb,
                      in_=b1_d.ap()[4 * D:].rearrange("(i p) -> p i", p=128))
    xsp_sb = pp.tile([128, 4, D], F32)
    nc.sync.dma_start(out=xsp_sb,
                      in_=xsp_d.ap().rearrange("(tb p) n -> p tb n", p=128))
    xo_all = pp.tile([128, 4, D], F32)   # attention-block output (resid in)

    a2a_in = dd.tile([NCORES, E2, TS], BF16)
    a2a_out = dd.tile([NCORES, E2, TS], BF16)

    # ================= Phase 0: RMSNorm(x) and transpose =================
    xn_dram = dd.tile([NTOK, D], BF16)
    attn = tc.tile_pool(name="attn", bufs=1)
    ap = attn.__enter__()
    xnT = ap.tile([128, NDC, NTOK], BF16)      # xnT[p, c, t] = xn[t, c*128+p]

    with tc.tile_pool(name="p0", bufs=3) as p0:
        for tt in range(NTOK // 128):
            xt = p0.tile([128, D], BF16, tag="xt")
            nc.sync.dma_start(out=xt, in_=xb_d.ap()[ts(tt, 128), :])
            sq = p0.tile([128, D], BF16, tag="sq")
            ssum = p0.tile([128, 1], F32, tag="ssum")
            nc.scalar.activation(out=sq, in_=xt, func=AF.Square,
                                 accum_out=ssum)
            rs = p0.tile([128, 1], F32, tag="rs")
            nc.scalar.activation(out=rs, in_=ssum, func=AF.Sqrt,
                                 bias=eps_t, scale=1.0 / D)
            nc.vector.reciprocal(out=rs, in_=rs)
            xn_t = p0.tile([128, D], BF16, tag="xnt")
            nc.vector.tensor_scalar_mul(out=xn_t, in0=xt, scalar1=rs)
            nc.sync.dma_start(out=xn_dram[ts(tt, 128), :], in_=xn_t)

    # transposed read-back: xnT tiles
    for c in range(NDC):
        for tt2 in range(NTOK // 512):
            nc.sync.dma_start(out=xnT[:, c, ts(tt2, 512)],
                              in_=xn_dram[ts(tt2, 512), ts(c, 128)],
                              transpose=True)

    # ================= Phase 1: QKV projections =================
    qT = ap.tile([128, NTOK], BF16)      # qT[e2, t_global]
    kT = ap.tile([128, NTOK], BF16)
    v0 = ap.tile([128, NTOK // 128, HD + 1], BF16)   # v + ones col, head 0
    v1 = ap.tile([128, NTOK // 128, HD + 1], BF16)
    nc.vector.memset(v0[:, :, HD:HD + 1], 1.0)
    nc.vector.memset(v1[:, :, HD:HD + 1], 1.0)

    wq_sb = ap.tile([128, NDC, E2], BF16)
    nc.sync.dma_start(out=wq_sb,
                      in_=wq_d.ap().rearrange("(c p) m -> p c m", p=128))
    wk_sb = ap.tile([128, NDC, E2], BF16)
    nc.sync.dma_start(out=wk_sb,
                      in_=wk_d.ap().rearrange("(c p) m -> p c m", p=128))
    wv_sb = ap.tile([128, NDC, E2], BF16)
    nc.sync.dma_start(out=wv_sb,
                      in_=wv_d.ap().rearrange("(c p) m -> p c m", p=128))
    mk_sb = ap.tile([128, 896], BF16)
    nc.sync.dma_start(out=mk_sb, in_=mk_d.ap())

    with tc.tile_pool(name="ps1", bufs=2, space="PSUM") as ps1:
        for tt in range(NTOK // 512):
            for w_sb, dstT in ((wq_sb, qT), (wk_sb, kT)):
                acc = ps1.tile([128, 512], F32, tag="qk")
                for c in range(NDC):
                    nc.tensor.matmul(acc, lhsT=w_sb[:, c, :],
                                     rhs=xnT[:, c, ts(tt, 512)],
                                     start=(c == 0), stop=(c == NDC - 1))
                nc.vector.tensor_copy(out=dstT[:, ts(tt, 512)], in_=acc)
        for gt in range(NTOK // 128):
            acc = ps1.tile([128, E2], F32, tag="v")
            for c in range(NDC):
                nc.tensor.matmul(acc, lhsT=xnT[:, c, ts(gt, 128)],
                                 rhs=wv_sb[:, c, :],
                                 start=(c == 0), stop=(c == NDC - 1))
            nc.vector.tensor_copy(out=v0[:, gt, 0:HD], in_=acc[:, 0:HD])
            nc.vector.tensor_copy(out=v1[:, gt, 0:HD], in_=acc[:, HD:E2])

    # ================= Phase 2: causal attention (2 heads, 2 batches) ======
    with tc.tile_pool(name="pa", bufs=1) as pa, \
         tc.tile_pool(name="psA", bufs=1, space="PSUM") as psA:
        for b in range(B):
            sums = pa.tile([8, 512], F32, tag=f"sums{b}")   # r = h*4 + tq
            recip_bf = pa.tile([8, 512], BF16, tag=f"recip{b}")
            for tq in range(T // 512):
                nblk = 4 * (tq + 1)
                q0 = b * T + tq * 512
                pe0 = pa.tile([128, nblk, 512], BF16, tag="pe0")
                pe1 = pa.tile([128, nblk, 512], BF16, tag="pe1")
                av0 = psA.tile([HD + 1, 512], F32, tag="av0")
                av1 = psA.tile([HD + 1, 512], F32, tag="av1")
                for blk in range(nblk):
                    k0 = b * T + blk * 128
                    s0 = psA.tile([128, 512], F32, tag="s0", bufs=2)
                    s1 = psA.tile([128, 512], F32, tag="s1", bufs=2)
                    nc.tensor.matmul(s0, lhsT=kT[0:HD, k0:k0 + 128],
                                     rhs=qT[0:HD, q0:q0 + 512])
                    nc.tensor.matmul(s1, lhsT=kT[HD:E2, k0:k0 + 128],
                                     rhs=qT[HD:E2, q0:q0 + 512])
                    nc.scalar.activation(out=pe0[:, blk, :], in_=s0,
                                         func=AF.Exp, scale=HD ** -0.5)
                    nc.scalar.activation(out=pe1[:, blk, :], in_=s1,
                                         func=AF.Exp, scale=HD ** -0.5)
                    if blk >= 4 * tq:               # diagonal block: mask
                        off = 384 - (blk - 4 * tq) * 128
                        nc.vector.tensor_mul(out=pe0[:, blk, :],
                                             in0=pe0[:, blk, :],
                                             in1=mk_sb[:, off:off + 512])
                        nc.vector.tensor_mul(out=pe1[:, blk, :],
                                             in0=pe1[:, blk, :],
                                             in1=mk_sb[:, off:off + 512])
                    gt = (b * T) // 128 + blk
                    nc.tensor.matmul(av0, lhsT=v0[:, gt, :],
                                     rhs=pe0[:, blk, :],
                                     start=(blk == 0), stop=(blk == nblk - 1))
                    nc.tensor.matmul(av1, lhsT=v1[:, gt, :],
                                     rhs=pe1[:, blk, :],
                                     start=(blk == 0), stop=(blk == nblk - 1))
                # move unnormalized o + sums to SBUF
                for h, av in ((0, av0), (1, av1)):
                    o_sb = pa.tile([HD + 1, 512], F32, tag=f"o{h}", bufs=5)
                    nc.scalar.copy(out=o_sb, in_=av)
                    r = h * 4 + tq
                    # cross-partition move of the sum row via SBUF->SBUF DMA
                    nc.sync.dma_start(out=sums[r:r + 1, :],
                                      in_=o_sb[HD:HD + 1, :])
                    _O_STASH[(b, tq, h)] = o_sb
            # after all tq of this batch: reciprocal + normalize + emit
            with nc.allow_low_precision(reason="softmax denom bcast in bf16"):
                nc.vector.reciprocal(out=recip_bf, in_=sums)
            for tq in range(T // 512):
                for h in range(HPC):
                    r = h * 4 + tq
                    bc = psA.tile([HD, 512], F32, tag="bc", bufs=2)
                    nc.tensor.matmul(bc, lhsT=sel8[:, r, :], rhs=recip_bf)
                    o_sb = _O_STASH.pop((b, tq, h))
                    chunk = pa.tile([HD, 512], BF16, tag="nrm", bufs=2)
                    nc.vector.tensor_mul(out=chunk, in0=o_sb[0:HD, :], in1=bc)
                    slot = b * 4 + tq
                    nc.sync.dma_start(
                        out=a2a_in[slot, h * HD:(h + 1) * HD, :], in_=chunk)

    attn.__exit__(None, None, None)

    # ================= Phase 3: AllToAll + output projection ==============
    nc.gpsimd.collective_compute(
        "AllToAll", ALU.bypass, replica_groups=[CORE_IDS],
        ins=[a2a_in[:].opt()], outs=[a2a_out[:].opt()])

    with tc.tile_pool(name="proj", bufs=1) as pj, \
         tc.tile_pool(name="ps3", bufs=4, space="PSUM") as ps3:
        oT_sb = pj.tile([128, NCORES, TS], BF16)
        nc.sync.dma_start(out=oT_sb,
                          in_=a2a_out[:].rearrange("s p f -> p s f"))
        wp_sb = pj.tile([128, NDC, D], BF16)
        nc.sync.dma_start(out=wp_sb,
                          in_=wp_d.ap().rearrange("(c p) n -> p c n", p=128))
        for tb in range(TS // 128):
            for dt in range(D // 512):
                acc = ps3.tile([128, 512], F32, tag="xo")
                for c in range(NDC):
                    nc.tensor.matmul(acc, lhsT=oT_sb[:, c, ts(tb, 128)],
                                     rhs=wp_sb[:, c, ts(dt, 512)],
                                     start=(c == 0), stop=(c == NDC - 1))
                nc.vector.tensor_add(out=xo_all[:, tb, ts(dt, 512)],
                                     in0=acc, in1=xsp_sb[:, tb, ts(dt, 512)])

    # ================= Phase 4: second RMSNorm + transpose ================
    xn2_dram = dd.tile([TS, D], BF16)
    ffn = tc.tile_pool(name="ffn", bufs=1)
    fp = ffn.__enter__()
    xn2T = fp.tile([128, NDC, TS], BF16)
    with tc.tile_pool(name="p4", bufs=3) as p4:
        for tb in range(TS // 128):
            sq = p4.tile([128, D], BF16, tag="sq4")
            ssum = p4.tile([128, 1], F32, tag="ssum4")
            nc.scalar.activation(out=sq, in_=xo_all[:, tb, :], func=AF.Square,
                                 accum_out=ssum)
            rs = p4.tile([128, 1], F32, tag="rs4")
            nc.scalar.activation(out=rs, in_=ssum, func=AF.Sqrt,
                                 bias=eps_t, scale=1.0 / D)
            nc.vector.reciprocal(out=rs, in_=rs)
            xn2_t = p4.tile([128, D], BF16, tag="xn2t")
            nc.vector.tensor_scalar_mul(out=xn2_t, in0=xo_all[:, tb, :],
                                        scalar1=rs)
            nc.sync.dma_start(out=xn2_dram[ts(tb, 128), :], in_=xn2_t)
    for c in range(NDC):
        nc.sync.dma_start(out=xn2T[:, c, :], in_=xn2_dram[:, ts(c, 128)],
                          transpose=True)

    # ================= Phase 5: GLU FFN first half =================
    gluT = fp.tile([128, 32, TS], BF16)   # gluT[p, i, t] = glu[t, i*128+p]
    with tc.tile_pool(name="p5", bufs=2) as p5, \
         tc.tile_pool(name="ps5", bufs=3, space="PSUM") as ps5:
        for qr in range(4):
            w1_t = p5.tile([128, NDC, 2048], BF16, tag="w1")
            nc.sync.dma_start(
                out=w1_t[:, :, 0:1024],
                in_=w1_d.ap()[:, qr * 1024:(qr + 1) * 1024]
                    .rearrange("(c p) m -> p c m", p=128))
            nc.sync.dma_start(
                out=w1_t[:, :, 1024:2048],
                in_=w1_d.ap()[:, 4 * D + qr * 1024:4 * D + (qr + 1) * 1024]
                    .rearrange("(c p) m -> p c m", p=128))
            for i_loc in range(8):
                ia = qr * 8 + i_loc
                ha = ps5.tile([128, TS], F32, tag="ha")
                for c in range(NDC):
                    nc.tensor.matmul(ha, lhsT=w1_t[:, c, ts(i_loc, 128)],
                                     rhs=xn2T[:, c, :],
                                     start=(c == 0), stop=(c == NDC - 1))
                aT = p5.tile([128, TS], BF16, tag="aT")
                nc.vector.tensor_scalar_add(out=aT, in0=ha,
                                            scalar1=b1a_sb[:, ia:ia + 1])
                hb = ps5.tile([128, TS], F32, tag="hb")
                for c in range(NDC):
                    nc.tensor.matmul(hb,
                                     lhsT=w1_t[:, c, 1024 + i_loc * 128:
                                               1024 + (i_loc + 1) * 128],
                                     rhs=xn2T[:, c, :],
                                     start=(c == 0), stop=(c == NDC - 1))
                sg = p5.tile([128, TS], BF16, tag="sg")
                nc.scalar.activation(out=sg, in_=hb, func=AF.Sigmoid,
                                     bias=b1b_sb[:, ia:ia + 1])
                nc.vector.tensor_mul(out=gluT[:, ia, :], in0=aT, in1=sg)

    # ================= Phase 6: down-projection + residual ================
    with tc.tile_pool(name="p6", bufs=2) as p6, \
         tc.tile_pool(name="ps6", bufs=1, space="PSUM") as ps6:
        accs = {}
        for tb in range(TS // 128):
            for dt in range(D // 512):
                accs[(tb, dt)] = ps6.tile([128, 512], F32, tag=f"y{tb}{dt}",
                                          name=f"y{tb}{dt}")
        for half in range(2):
            w2_t = p6.tile([128, 16, D], BF16, tag="w2")
            nc.sync.dma_start(
                out=w2_t,
                in_=w2_d.ap()[half * 2048:(half + 1) * 2048, :]
                    .rearrange("(i p) n -> p i n", p=128))
            for tb in range(TS // 128):
                for dt in range(D // 512):
                    for i_loc in range(16):
                        i = half * 16 + i_loc
                        nc.tensor.matmul(
                            accs[(tb, dt)],
                            lhsT=gluT[:, i, ts(tb, 128)],
                            rhs=w2_t[:, i_loc, ts(dt, 512)],
                            start=(half == 0 and i_loc == 0), stop=False)
        for tb in range(TS // 128):
            for dt in range(D // 512):
                acc = accs[(tb, dt)]
                nc.tensor.matmul(acc, lhsT=ones1, rhs=b2_sb[:, ts(dt, 512)],
                                 start=False, stop=True)
                out_t = p6.tile([128, 512], F32, tag="out", bufs=3)
                nc.vector.tensor_add(out=out_t, in0=acc,
                                     in1=xo_all[:, tb, ts(dt, 512)])
                nc.sync.dma_start(out=out_d.ap()[ts(tb, 128), ts(dt, 512)],
                                  in_=out_t)

    ffn.__exit__(None, None, None)
    dram.__exit__(None, None, None)
    persist.__exit__(None, None, None)


def _prep_inputs(x, wq, wk, wv, w_proj, b_proj, w1, b1, w2, b2, g1, g2):
    bf16 = ml_dtypes.bfloat16
    xf = np.asarray(x, np.float32).reshape(NTOK, D)
    g1 = np.asarray(g1, np.float32)
    g2 = np.asarray(g2, np.float32)
    wqf = (np.asarray(wq, np.float32) * g1[None, :, None]).astype(bf16)
    wkf = (np.asarray(wk, np.float32) * g1[None, :, None]).astype(bf16)
    wvf = (np.asarray(wv, np.float32) * g1[None, :, None]).astype(bf16)
    w1f = np.ascontiguousarray(
        (np.asarray(w1, np.float32) * g2[:, None]).astype(bf16))
    w2b = np.ascontiguousarray(np.asarray(w2, np.float32).astype(bf16))
    wpb = np.ascontiguousarray(np.asarray(w_proj, np.float32).astype(bf16))
    b2r = np.ascontiguousarray(
        np.asarray(b2, np.float32).astype(bf16).reshape(1, D))
    b1f = np.ascontiguousarray(np.asarray(b1, np.float32))
    bp = np.asarray(b_proj, np.float32)
    xb = np.ascontiguousarray(xf.astype(bf16))
    mk = np.ascontiguousarray(
        (np.arange(128)[:, None] <= (np.arange(896)[None, :] - 384))
        .astype(bf16))
    sel = np.ascontiguousarray(
        np.broadcast_to(np.eye(8, dtype=bf16)[:, :, None], (8, 8, 64)))

    in_maps = []
    for c in range(NCORES):
        h0, h1 = HPC * c, HPC * c + 1
        in_maps.append({
            "xb": xb,
            "wq": np.ascontiguousarray(np.concatenate([wqf[h0], wqf[h1]], 1)),
            "wk": np.ascontiguousarray(np.concatenate([wkf[h0], wkf[h1]], 1)),
            "wv": np.ascontiguousarray(np.concatenate([wvf[h0], wvf[h1]], 1)),
            "wp": wpb,
            "w1": w1f,
            "b1": b1f,
            "w2": w2b,
            "b2": b2r,
            "xsp": np.ascontiguousarray(xf[TS * c:TS * (c + 1)] + bp[None, :]),
            "mk": mk,
            "sel": sel,
        })
    return in_maps


def kernel(**inputs):
    in_maps = _prep_inputs(**inputs)
    if "nc" not in _CACHE:
        _CACHE["nc"] = _build()
    res = run_bass_kernel_spmd(_CACHE["nc"], in_maps, CORE_IDS)
    out = np.concatenate([res.results[c]["out"] for c in range(NCORES)], 0)
    return out.reshape(B, T, D).astype(np.float32)


if __name__ == "__main__":
    import reference
    inputs = {k: np.asarray(v) for k, v in reference.setup_inputs().items()}
    got = kernel(**inputs)
    want = np.asarray(reference.reference(**inputs))
    err = np.abs(got - want)
    scale = np.abs(want).max()
    print("max abs err:", err.max(), "scale:", scale)
    print("rel err (max/scale):", err.max() / scale)


# revision 7
# speedup vs baseline: 47.5008x; 47.5008x over previous
"""Trainium2 Bass kernel for a dense transformer block (RMSNorm -> causal MHA
-> residual -> RMSNorm -> GLU FFN -> residual), SPMD across 8 NeuronCores.

Sharding: tensor-parallel attention (2 heads/core) -> AllToAll of per-head
attention outputs -> token-parallel proj + FFN (512 tokens/core).

All matmuls run in bf16 with fp32 PSUM accumulation; norms/softmax math in fp32.
"""
import numpy as np
import ml_dtypes

import concourse.bass as bass
import concourse.tile as tile
from concourse import bacc, mybir
from concourse.bass_utils import run_bass_kernel_spmd

F32 = mybir.dt.float32
BF16 = mybir.dt.bfloat16
AF = mybir.ActivationFunctionType
ALU = mybir.AluOpType

B, T, D, H, HD = 2, 2048, 1024, 16, 64
NCORES = 8
HPC = H // NCORES          # heads per core = 2
E2 = HPC * HD              # per-core attention channels = 128
NTOK = B * T               # 4096
TS = NTOK // NCORES        # tokens per core slice = 512
EPS = 1e-6
NDC = D // 128             # 8 D-chunks
CORE_IDS = list(range(NCORES))

_CACHE = {}
_O_STASH = {}


def _build(reps=1):
    nc = bacc.Bacc("TRN2", target_bir_lowering=False, debug=False,
                   num_devices=NCORES)

    # ---- DRAM I/O ----
    tensors = dict(
        xb=nc.dram_tensor("xb", [NTOK, D], BF16, kind="ExternalInput"),
        wq=nc.dram_tensor("wq", [D, E2], BF16, kind="ExternalInput"),
        wk=nc.dram_tensor("wk", [D, E2], BF16, kind="ExternalInput"),
        wv=nc.dram_tensor("wv", [D, E2], BF16, kind="ExternalInput"),
        wp=nc.dram_tensor("wp", [D, D], BF16, kind="ExternalInput"),
        w1=nc.dram_tensor("w1", [D, 8 * D], BF16, kind="ExternalInput"),
        b1=nc.dram_tensor("b1", [8 * D], F32, kind="ExternalInput"),
        w2=nc.dram_tensor("w2", [4 * D, D], BF16, kind="ExternalInput"),
        b2=nc.dram_tensor("b2", [1, D], BF16, kind="ExternalInput"),
        xsp=nc.dram_tensor("xsp", [TS, D], F32, kind="ExternalInput"),
        mk=nc.dram_tensor("mk", [128, 896], BF16, kind="ExternalInput"),
        sel=nc.dram_tensor("sel", [8, 8, 64], BF16, kind="ExternalInput"),
        out=nc.dram_tensor("out", [TS, D], F32, kind="ExternalOutput"),
    )

    with tile.TileContext(nc) as tc:
        for _ in range(reps):
            _body(nc, tc, tensors)
    nc.compile()
    return nc


def _body(nc, tc, tn):
    ts = bass.ts
    xb_d, wq_d, wk_d, wv_d, wp_d = tn["xb"], tn["wq"], tn["wk"], tn["wv"], tn["wp"]
    w1_d, b1_d, w2_d, b2_d = tn["w1"], tn["b1"], tn["w2"], tn["b2"]
    xsp_d, mk_d, out_d = tn["xsp"], tn["mk"], tn["out"]
    sel_d = tn["sel"]

    persist = tc.tile_pool(name="persist", bufs=1)
    pp = persist.__enter__()
    dram = tc.tile_pool(name="dram", bufs=1, space="DRAM")
    dd = dram.__enter__()

    # persistent small tensors
    eps_t = pp.tile([128, 1], F32)
    nc.vector.memset(eps_t, EPS)
    ones1 = pp.tile([1, 128], BF16)
    nc.vector.memset(ones1, 1.0)
    sel8 = pp.tile([8, 8, HD], BF16)      # sel8[k, r, :] = (k == r)
    nc.sync.dma_start(out=sel8, in_=sel_d.ap())
    b2_sb = pp.tile([1, D], BF16)
    nc.sync.dma_start(out=b2_sb, in_=b2_d.ap())
    b1a_sb = pp.tile([128, 32], F32)
    nc.sync.dma_start(out=b1a_sb,
                      in_=b1_d.ap()[:4 * D].rearrange("(i p) -> p i", p=128))
    b1b_sb = pp.tile([128, 32], F32)
    nc.sync.dma_start(out=b1b_sb,
                      in_=b1_d.ap()[4 * D:].rearrange("(i p) -> p i", p=128))
    xsp_sb = pp.tile([128, 4, D], F32)
    nc.sync.dma_start(out=xsp_sb,
                      in_=xsp_d.ap().rearrange("(tb p) n -> p tb n", p=128))
    xo_all = pp.tile([128, 4, D], F32)   # attention-block output (resid in)

    a2a_in = dd.tile([NCORES, E2, TS], BF16)
    a2a_out = dd.tile([NCORES, E2, TS], BF16)

    # ================= Phase 0: RMSNorm(x) and transpose =================
    xn_dram = dd.tile([NTOK, D], BF16)
    attn = tc.tile_pool(name="attn", bufs=1)
    ap = attn.__enter__()
    xnT = ap.tile([128, NDC, NTOK], BF16)      # xnT[p, c, t] = xn[t, c*128+p]

    with tc.tile_pool(name="p0", bufs=3) as p0:
        for tt in range(NTOK // 128):
            xt = p0.tile([128, D], BF16, tag="xt")
            nc.sync.dma_start(out=xt, in_=xb_d.ap()[ts(tt, 128), :])
            sq = p0.tile([128, D], BF16, tag="sq")
            ssum = p0.tile([128, 1], F32, tag="ssum")
            nc.scalar.activation(out=sq, in_=xt, func=AF.Square,
                                 accum_out=ssum)
            rs = p0.tile([128, 1], F32, tag="rs")
            nc.scalar.activation(out=rs, in_=ssum, func=AF.Sqrt,
                                 bias=eps_t, scale=1.0 / D)
            nc.vector.reciprocal(out=rs, in_=rs)
            xn_t = p0.tile([128, D], BF16, tag="xnt")
            nc.vector.tensor_scalar_mul(out=xn_t, in0=xt, scalar1=rs)
            nc.sync.dma_start(out=xn_dram[ts(tt, 128), :], in_=xn_t)

    # transposed read-back: xnT tiles
    for c in range(NDC):
        for tt2 in range(NTOK // 512):
            nc.sync.dma_start(out=xnT[:, c, ts(tt2, 512)],
                              in_=xn_dram[ts(tt2, 512), ts(c, 128)],
                              transpose=True)

    # ================= Phase 1: QKV projections =================
    qT = ap.tile([128, NTOK], BF16)      # qT[e2, t_global]
    kT = ap.tile([128, NTOK], BF16)
    v0 = ap.tile([128, NTOK // 128, HD + 1], BF16)   # v + ones col, head 0
    v1 = ap.tile([128, NTOK // 128, HD + 1], BF16)
    nc.vector.memset(v0[:, :, HD:HD + 1], 1.0)
    nc.vector.memset(v1[:, :, HD:HD + 1], 1.0)

    wq_sb = ap.tile([128, NDC, E2], BF16)
    nc.sync.dma_start(out=wq_sb,
                      in_=wq_d.ap().rearrange("(c p) m -> p c m", p=128))
    wk_sb = ap.tile([128, NDC, E2], BF16)
    nc.sync.dma_start(out=wk_sb,
                      in_=wk_d.ap().rearrange("(c p) m -> p c m", p=128))
    wv_sb = ap.tile([128, NDC, E2], BF16)
    nc.sync.dma_start(out=wv_sb,
                      in_=wv_d.ap().rearrange("(c p) m -> p c m", p=128))
    mk_sb = ap.tile([128, 896], BF16)
    nc.sync.dma_start(out=mk_sb, in_=mk_d.ap())

    with tc.tile_pool(name="ps1", bufs=2, space="PSUM") as ps1:
        for tt in range(NTOK // 512):
            for w_sb, dstT in ((wq_sb, qT), (wk_sb, kT)):
                acc = ps1.tile([128, 512], F32, tag="qk")
                for c in range(NDC):
                    nc.tensor.matmul(acc, lhsT=w_sb[:, c, :],
                                     rhs=xnT[:, c, ts(tt, 512)],
                                     start=(c == 0), stop=(c == NDC - 1))
                nc.vector.tensor_copy(out=dstT[:, ts(tt, 512)], in_=acc)
        for gt in range(NTOK // 128):
            acc = ps1.tile([128, E2], F32, tag="v")
            for c in range(NDC):
                nc.tensor.matmul(acc, lhsT=xnT[:, c, ts(gt, 128)],
                                 rhs=wv_sb[:, c, :],
                                 start=(c == 0), stop=(c == NDC - 1))
            nc.vector.tensor_copy(out=v0[:, gt, 0:HD], in_=acc[:, 0:HD])
            nc.vector.tensor_copy(out=v1[:, gt, 0:HD], in_=acc[:, HD:E2])

    # ================= Phase 2: causal attention (2 heads, 2 batches) ======
    with tc.tile_pool(name="pa", bufs=1) as pa, \
         tc.tile_pool(name="psA", bufs=1, space="PSUM") as psA:
        for b in range(B):
            sums = pa.tile([8, 512], F32, tag=f"sums{b}")   # r = h*4 + tq
            recip_bf = pa.tile([8, 512], BF16, tag=f"recip{b}")
            for tq in range(T // 512):
                nblk = 4 * (tq + 1)
                q0 = b * T + tq * 512
                pe0 = pa.tile([128, nblk, 512], BF16, tag="pe0")
                pe1 = pa.tile([128, nblk, 512], BF16, tag="pe1")
                av0 = psA.tile([HD + 1, 512], F32, tag="av0")
                av1 = psA.tile([HD + 1, 512], F32, tag="av1")
                for blk in range(nblk):
                    k0 = b * T + blk * 128
                    s0 = psA.tile([128, 512], F32, tag="s0", bufs=2)
                    s1 = psA.tile([128, 512], F32, tag="s1", bufs=2)
                    nc.tensor.matmul(s0, lhsT=kT[0:HD, k0:k0 + 128],
                                     rhs=qT[0:HD, q0:q0 + 512])
                    nc.tensor.matmul(s1, lhsT=kT[HD:E2, k0:k0 + 128],
                                     rhs=qT[HD:E2, q0:q0 + 512])
                    nc.scalar.activation(out=pe0[:, blk, :], in_=s0,
                                         func=AF.Exp, scale=HD ** -0.5)
                    nc.scalar.activation(out=pe1[:, blk, :], in_=s1,
                                         func=AF.Exp, scale=HD ** -0.5)
                    if blk >= 4 * tq:               # diagonal block: mask
                        off = 384 - (blk - 4 * tq) * 128
                        nc.vector.tensor_mul(out=pe0[:, blk, :],
                                             in0=pe0[:, blk, :],
                                             in1=mk_sb[:, off:off + 512])
                        nc.vector.tensor_mul(out=pe1[:, blk, :],
                                             in0=pe1[:, blk, :],
                                             in1=mk_sb[:, off:off + 512])
                    gt = (b * T) // 128 + blk
                    nc.tensor.matmul(av0, lhsT=v0[:, gt, :],
                                     rhs=pe0[:, blk, :],
                                     start=(blk == 0), stop=(blk == nblk - 1))
                    nc.tensor.matmul(av1, lhsT=v1[:, gt, :],
                                     rhs=pe1[:, blk, :],
                                     start=(blk == 0), stop=(blk == nblk - 1))
                # move unnormalized o + sums to SBUF
                for h, av in ((0, av0), (1, av1)):
                    o_sb = pa.tile([HD + 1, 512], F32, tag=f"o{h}", bufs=5)
                    nc.scalar.copy(out=o_sb, in_=av)
                    r = h * 4 + tq
                    # cross-partition move of the sum row via SBUF->SBUF DMA
                    nc.sync.dma_start(out=sums[r:r + 1, :],
                                      in_=o_sb[HD:HD + 1, :])
                    _O_STASH[(b, tq, h)] = o_sb
            # after all tq of this batch: reciprocal + normalize + emit
            with nc.allow_low_precision(reason="softmax denom bcast in bf16"):
                nc.vector.reciprocal(out=recip_bf, in_=sums)
            for tq in range(T // 512):
                for h in range(HPC):
                    r = h * 4 + tq
                    bc = psA.tile([HD, 512], F32, tag="bc", bufs=2)
                    nc.tensor.matmul(bc, lhsT=sel8[:, r, :], rhs=recip_bf)
                    o_sb = _O_STASH.pop((b, tq, h))
                    chunk = pa.tile([HD, 512], BF16, tag="nrm", bufs=2)
                    nc.vector.tensor_mul(out=chunk, in0=o_sb[0:HD, :], in1=bc)
                    slot = b * 4 + tq
                    nc.sync.dma_start(
                        out=a2a_in[slot, h * HD:(h + 1) * HD, :], in_=chunk)

    attn.__exit__(None, None, None)

    # ================= Phase 3: AllToAll + output projection ==============
    nc.gpsimd.collective_compute(
        "AllToAll", ALU.bypass, replica_groups=[CORE_IDS],
        ins=[a2a_in[:].opt()], outs=[a2a_out[:].opt()])

    with tc.tile_pool(name="proj", bufs=1) as pj, \
         tc.tile_pool(name="ps3", bufs=4, space="PSUM") as ps3:
        oT_sb = pj.tile([128, NCORES, TS], BF16)
        nc.sync.dma_start(out=oT_sb,
                          in_=a2a_out[:].rearrange("s p f -> p s f"))
        wp_sb = pj.tile([128, NDC, D], BF16)
        nc.sync.dma_start(out=wp_sb,
                          in_=wp_d.ap().rearrange("(c p) n -> p c n", p=128))
        for tb in range(TS // 128):
            for dt in range(D // 512):
                acc = ps3.tile([128, 512], F32, tag="xo")
                for c in range(NDC):
                    nc.tensor.matmul(acc, lhsT=oT_sb[:, c, ts(tb, 128)],
                                     rhs=wp_sb[:, c, ts(dt, 512)],
                                     start=(c == 0), stop=(c == NDC - 1))
                nc.vector.tensor_add(out=xo_all[:, tb, ts(dt, 512)],
                                     in0=acc, in1=xsp_sb[:, tb, ts(dt, 512)])

    # ================= Phase 4: second RMSNorm + transpose ================
    xn2_dram = dd.tile([TS, D], BF16)
    ffn = tc.tile_pool(name="ffn", bufs=1)
    fp = ffn.__enter__()
    xn2T = fp.tile([128, NDC, TS], BF16)
    with tc.tile_pool(name="p4", bufs=3) as p4:
        for tb in range(TS // 128):
            sq = p4.tile([128, D], BF16, tag="sq4")
            ssum = p4.tile([128, 1], F32, tag="ssum4")
            nc.scalar.activation(out=sq, in_=xo_all[:, tb, :], func=AF.Square,
                                 accum_out=ssum)
            rs = p4.tile([128, 1], F32, tag="rs4")
            nc.scalar.activation(out=rs, in_=ssum, func=AF.Sqrt,
                                 bias=eps_t, scale=1.0 / D)
            nc.vector.reciprocal(out=rs, in_=rs)
            xn2_t = p4.tile([128, D], BF16, tag="xn2t")
            nc.vector.tensor_scalar_mul(out=xn2_t, in0=xo_all[:, tb, :],
                                        scalar1=rs)
            nc.sync.dma_start(out=xn2_dram[ts(tb, 128), :], in_=xn2_t)
    for c in range(NDC):
        nc.sync.dma_start(out=xn2T[:, c, :], in_=xn2_dram[:, ts(c, 128)],
                          transpose=True)

    # ================= Phase 5: GLU FFN first half =================
    gluT = fp.tile([128, 32, TS], BF16)   # gluT[p, i, t] = glu[t, i*128+p]
    with tc.tile_pool(name="p5", bufs=2) as p5, \
         tc.tile_pool(name="ps5", bufs=3, space="PSUM") as ps5:
        for qr in range(4):
            w1_t = p5.tile([128, NDC, 2048], BF16, tag="w1")
            nc.sync.dma_start(
                out=w1_t[:, :, 0:1024],
                in_=w1_d.ap()[:, qr * 1024:(qr + 1) * 1024]
                    .rearrange("(c p) m -> p c m", p=128))
            nc.sync.dma_start(
                out=w1_t[:, :, 1024:2048],
                in_=w1_d.ap()[:, 4 * D + qr * 1024:4 * D + (qr + 1) * 1024]
                    .rearrange("(c p) m -> p c m", p=128))
            for i_loc in range(8):
                ia = qr * 8 + i_loc
                ha = ps5.tile([128, TS], F32, tag="ha")
                for c in range(NDC):
                    nc.tensor.matmul(ha, lhsT=w1_t[:, c, ts(i_loc, 128)],
                                     rhs=xn2T[:, c, :],
                                     start=(c == 0), stop=(c == NDC - 1))
                aT = p5.tile([128, TS], BF16, tag="aT")
                nc.vector.tensor_scalar_add(out=aT, in0=ha,
                                            scalar1=b1a_sb[:, ia:ia + 1])
                hb = ps5.tile([128, TS], F32, tag="hb")
                for c in range(NDC):
                    nc.tensor.matmul(hb,
                                     lhsT=w1_t[:, c, 1024 + i_loc * 128:
                                               1024 + (i_loc + 1) * 128],
                                     rhs=xn2T[:, c, :],
                                     start=(c == 0), stop=(c == NDC - 1))
                sg = p5.tile([128, TS], BF16, tag="sg")
                nc.scalar.activation(out=sg, in_=hb, func=AF.Sigmoid,
                                     bias=b1b_sb[:, ia:ia + 1])
                nc.vector.tensor_mul(out=gluT[:, ia, :], in0=aT, in1=sg)

    # ================= Phase 6: down-projection + residual ================
    with tc.tile_pool(name="p6", bufs=2) as p6, \
         tc.tile_pool(name="ps6", bufs=1, space="PSUM") as ps6:
        accs = {}
        for tb in range(TS // 128):
            for dt in range(D // 512):
                accs[(tb, dt)] = ps6.tile([128, 512], F32, tag=f"y{tb}{dt}",
                                          name=f"y{tb}{dt}")
        for half in range(2):
            w2_t = p6.tile([128, 16, D], BF16, tag="w2")
            nc.sync.dma_start(
                out=w2_t,
                in_=w2_d.ap()[half * 2048:(half + 1) * 2048, :]
                    .rearrange("(i p) n -> p i n", p=128))
            for tb in range(TS // 128):
                for dt in range(D // 512):
                    for i_loc in range(16):
                        i = half * 16 + i_loc
                        nc.tensor.matmul(
                            accs[(tb, dt)],
                            lhsT=gluT[:, i, ts(tb, 128)],
                            rhs=w2_t[:, i_loc, ts(dt, 512)],
                            start=(half == 0 and i_loc == 0), stop=False)
        for tb in range(TS // 128):
            for dt in range(D // 512):
                acc = accs[(tb, dt)]
                nc.tensor.matmul(acc, lhsT=ones1, rhs=b2_sb[:, ts(dt, 512)],
                                 start=False, stop=True)
                out_t = p6.tile([128, 512], F32, tag="out", bufs=3)
                nc.vector.tensor_add(out=out_t, in0=acc,
                                     in1=xo_all[:, tb, ts(dt, 512)])
                nc.sync.dma_start(out=out_d.ap()[ts(tb, 128), ts(dt, 512)],
                                  in_=out_t)

    ffn.__exit__(None, None, None)
    dram.__exit__(None, None, None)
    persist.__exit__(None, None, None)


def _prep_inputs(x, wq, wk, wv, w_proj, b_proj, w1, b1, w2, b2, g1, g2):
    bf16 = ml_dtypes.bfloat16
    xf = np.asarray(x, np.float32).reshape(NTOK, D)
    g1 = np.asarray(g1, np.float32)
    g2 = np.asarray(g2, np.float32)
    wqf = (np.asarray(wq, np.float32) * g1[None, :, None]).astype(bf16)
    wkf = (np.asarray(wk, np.float32) * g1[None, :, None]).astype(bf16)
    wvf = (np.asarray(wv, np.float32) * g1[None, :, None]).astype(bf16)
    w1f = np.ascontiguousarray(
        (np.asarray(w1, np.float32) * g2[:, None]).astype(bf16))
    w2b = np.ascontiguousarray(np.asarray(w2, np.float32).astype(bf16))
    wpb = np.ascontiguousarray(np.asarray(w_proj, np.float32).astype(bf16))
    b2r = np.ascontiguousarray(
        np.asarray(b2, np.float32).astype(bf16).reshape(1, D))
    b1f = np.ascontiguousarray(np.asarray(b1, np.float32))
    bp = np.asarray(b_proj, np.float32)
    xb = np.ascontiguousarray(xf.astype(bf16))
    mk = np.ascontiguousarray(
        (np.arange(128)[:, None] <= (np.arange(896)[None, :] - 384))
        .astype(bf16))
    sel = np.ascontiguousarray(
        np.broadcast_to(np.eye(8, dtype=bf16)[:, :, None], (8, 8, 64)))

    in_maps = []
    for c in range(NCORES):
        h0, h1 = HPC * c, HPC * c + 1
        in_maps.append({
            "xb": xb,
            "wq": np.ascontiguousarray(np.concatenate([wqf[h0], wqf[h1]], 1)),
            "wk": np.ascontiguousarray(np.concatenate([wkf[h0], wkf[h1]], 1)),
            "wv": np.ascontiguousarray(np.concatenate([wvf[h0], wvf[h1]], 1)),
            "wp": wpb,
            "w1": w1f,
            "b1": b1f,
            "w2": w2b,
            "b2": b2r,
            "xsp": np.ascontiguousarray(xf[TS * c:TS * (c + 1)] + bp[None, :]),
            "mk": mk,
            "sel": sel,
        })
    return in_maps


def kernel(**inputs):
    in_maps = _prep_inputs(**inputs)
    if "nc" not in _CACHE:
        _CACHE["nc"] = _build()
    res = run_bass_kernel_spmd(_CACHE["nc"], in_maps, CORE_IDS)
    out = np.concatenate([res.results[c]["out"] for c in range(NCORES)], 0)
    return out.reshape(B, T, D).astype(np.float32)


if __name__ == "__main__":
    import reference
    inputs = {k: np.asarray(v) for k, v in reference.setup_inputs().items()}
    got = kernel(**inputs)
    want = np.asarray(reference.reference(**inputs))
    err = np.abs(got - want)
    scale = np.abs(want).max()
    print("max abs err:", err.max(), "scale:", scale)
    print("rel err (max/scale):", err.max() / scale)


# revision 24
# speedup vs baseline: 183.3763x; 3.8605x over previous
"""Trainium2 Bass kernel for a dense transformer block (RMSNorm -> causal MHA
-> residual -> RMSNorm -> GLU FFN -> residual), SPMD across 8 NeuronCores.

Sharding: tensor-parallel attention (2 heads/core) -> AllToAll of per-head
attention outputs -> token-parallel proj + FFN (512 tokens/core).

All matmuls run in bf16 with fp32 PSUM accumulation; norms/softmax math in
fp32. The host supplies x pre-transposed (x.T) so no on-device transposes are
needed for the attention block; RMS scales are folded into the QKV epilogues.
"""
import numpy as np
import ml_dtypes

import concourse.bass as bass
import concourse.tile as tile
from concourse import bacc, mybir
from concourse.bass_utils import run_bass_kernel_spmd

F32 = mybir.dt.float32
BF16 = mybir.dt.bfloat16
AF = mybir.ActivationFunctionType
ALU = mybir.AluOpType

B, T, D, H, HD = 2, 2048, 1024, 16, 64
NCORES = 8
HPC = H // NCORES          # heads per core = 2
E2 = HPC * HD              # per-core attention channels = 128
NTOK = B * T               # 4096
TS = NTOK // NCORES        # tokens per core slice = 512
EPS = 1e-6
NDC = D // 128             # 8 D-chunks
CORE_IDS = list(range(NCORES))

_CACHE = {}
_O_STASH = {}
_W2Q = {}


def _build(reps=1, variant="full", loop_k=0, stop_after=99):
    nc = bacc.Bacc("TRN2", target_bir_lowering=False, debug=False,
                   num_devices=NCORES)

    tensors = dict(
        xt=nc.dram_tensor("xt", [D, NTOK], BF16, kind="ExternalInput"),
        wq=nc.dram_tensor("wq", [D, E2], BF16, kind="ExternalInput"),
        wk=nc.dram_tensor("wk", [D, E2], BF16, kind="ExternalInput"),
        wv=nc.dram_tensor("wv", [D, E2], BF16, kind="ExternalInput"),
        wp=nc.dram_tensor("wp", [D, D], BF16, kind="ExternalInput"),
        w1=nc.dram_tensor("w1", [D, 8 * D], BF16, kind="ExternalInput"),
        b1=nc.dram_tensor("b1", [8 * D], F32, kind="ExternalInput"),
        b1r=nc.dram_tensor("b1r", [4 * D], BF16, kind="ExternalInput"),
        w2=nc.dram_tensor("w2", [4 * D, D], BF16, kind="ExternalInput"),
        b2=nc.dram_tensor("b2", [1, D], BF16, kind="ExternalInput"),
        xsp=nc.dram_tensor("xsp", [TS, D], F32, kind="ExternalInput"),
        mk=nc.dram_tensor("mk", [128, 896], BF16, kind="ExternalInput"),
        sel=nc.dram_tensor("sel", [8, 8, 64], BF16, kind="ExternalInput"),
        osel=nc.dram_tensor("osel", [128, 8, 8], BF16, kind="ExternalInput"),
        bsel=nc.dram_tensor("bsel", [8, 8, 128], BF16, kind="ExternalInput"),
        out=nc.dram_tensor("out", [TS, D], F32, kind="ExternalOutput"),
    )

    with tile.TileContext(nc) as tc:
        if loop_k:
            with tc.For_i(0, loop_k, 1):
                _body(nc, tc, tensors, variant=variant, stop_after=stop_after)
        else:
            for _ in range(reps):
                _body(nc, tc, tensors, variant=variant, stop_after=stop_after)
    nc.compile()
    return nc


def _finish(*pools):
    for p in pools:
        p.__exit__(None, None, None)


def _body(nc, tc, tn, variant="full", stop_after=99):
    ts = bass.ts
    xt_d, wq_d, wk_d, wv_d, wp_d = tn["xt"], tn["wq"], tn["wk"], tn["wv"], tn["wp"]
    w1_d, b1_d, w2_d, b2_d = tn["w1"], tn["b1"], tn["w2"], tn["b2"]
    xsp_d, mk_d, out_d = tn["xsp"], tn["mk"], tn["out"]
    sel_d, osel_d, bsel_d, b1r_d = tn["sel"], tn["osel"], tn["bsel"], tn["b1r"]

    persist = tc.tile_pool(name="persist", bufs=1)
    pp = persist.__enter__()
    dram = tc.tile_pool(name="dram", bufs=1, space="DRAM")
    dd = dram.__enter__()

    # ---- persistent small tensors ----
    eps_t = pp.tile([128, 1], F32)
    nc.vector.memset(eps_t, EPS)
    ones1 = pp.tile([1, 128], BF16)
    nc.vector.memset(ones1, 1.0)
    b1a_sb = pp.tile([128, 32], F32)
    nc.sync.dma_start(out=b1a_sb,
                      in_=b1_d.ap()[:4 * D].rearrange("(i p) -> p i", p=128))
    sel8 = pp.tile([8, 8, HD], BF16)      # sel8[k, r, :] = (k == r)
    nc.sync.dma_start(out=sel8, in_=sel_d.ap())
    b2_sb = pp.tile([1, D], BF16)
    nc.sync.dma_start(out=b2_sb, in_=b2_d.ap())
    b1b_sb = pp.tile([128, 32], F32)
    nc.sync.dma_start(out=b1b_sb,
                      in_=b1_d.ap()[4 * D:].rearrange("(i p) -> p i", p=128))
    # xsp_sb carries residual state through the whole kernel:
    # x+b_proj -> (phase 3, in place) attention-block output xo
    # -> (FFN half 0, in place) xo + y_half0 + b2 -> final out = that + y_half1
    xsp_sb = pp.tile([128, 4, D], F32)
    nc.sync.dma_start(out=xsp_sb,
                      in_=xsp_d.ap().rearrange("(tb p) n -> p tb n", p=128))

    a2a_in = dd.tile([NCORES, E2, TS], BF16)
    a2a_out = dd.tile([NCORES, E2, TS], BF16)

    # ===== Phase 0: RMS stats in transposed layout (x.T supplied by host) ===
    attn = tc.tile_pool(name="attn", bufs=1)
    ap = attn.__enter__()
    psm_cm = tc.tile_pool(name="psM", bufs=1, space="PSUM")
    psM = psm_cm.__enter__()

    xnT = ap.tile([128, NDC, NTOK], BF16)      # x.T (unscaled)
    nc.sync.dma_start(out=xnT,
                      in_=xt_d.ap().rearrange("(c p) t -> p c t", p=128))
    osel_sb = ap.tile([128, 8, 8], BF16)
    nc.sync.dma_start(out=osel_sb, in_=osel_d.ap())
    bsel_sb = ap.tile([8, 8, 128], BF16)
    nc.sync.dma_start(out=bsel_sb, in_=bsel_d.ap())

    sums8 = psM.tile([8, 512], F32, tag="small", bufs=2)
    first = True
    for r in range(NTOK // 512):
        for c in range(NDC):
            sqt = ap.tile([128, 512], BF16, tag="sqt", bufs=2)
            nc.vector.tensor_mul(out=sqt, in0=xnT[:, c, ts(r, 512)],
                                 in1=xnT[:, c, ts(r, 512)])
            nc.tensor.matmul(sums8, lhsT=osel_sb[:, r, :], rhs=sqt,
                             start=first,
                             stop=(r == NTOK // 512 - 1 and c == NDC - 1))
            first = False
    rstile = ap.tile([8, 512], F32)
    nc.scalar.activation(out=rstile, in_=sums8, func=AF.Sqrt,
                         bias=eps_t[0:8], scale=1.0 / D)
    nscale_f = ap.tile([8, 512], F32)
    nc.vector.reciprocal(out=nscale_f, in_=rstile)
    nscale = ap.tile([8, 512], BF16)
    nc.vector.tensor_copy(out=nscale, in_=nscale_f)
    nbc_all = ap.tile([128, 8, 512], BF16)   # rms scale bcast per t-chunk
    for r in range(NTOK // 512):
        nbc = psM.tile([128, 512], F32, tag="small", bufs=2)
        nc.tensor.matmul(nbc, lhsT=bsel_sb[:, r, :], rhs=nscale)
        nc.vector.tensor_copy(out=nbc_all[:, r, :], in_=nbc)
    # per-token scale in partition-major layout for scaling V rows
    nsc_dram = dd.tile([8, 512], F32)
    nc.gpsimd.dma_start(out=nsc_dram[:], in_=nscale_f)
    scale_t = ap.tile([128, 32], F32)
    nc.gpsimd.dma_start(
        out=scale_t,
        in_=nsc_dram[:].rearrange("r (g p) -> p (r g)", p=128))

    if stop_after < 1:
        _finish(psm_cm, attn, dram, persist)
        return

    # ================= Phase 1: QKV projections =================
    qT = ap.tile([128, NTOK], BF16)      # qT[e2, t_global], rms-scaled
    kT = ap.tile([128, NTOK], BF16)
    v0 = ap.tile([128, NTOK // 128, HD + 1], BF16)   # v + ones col, head 0
    v1 = ap.tile([128, NTOK // 128, HD + 1], BF16)
    nc.vector.memset(v0[:, :, HD:HD + 1], 1.0)
    nc.vector.memset(v1[:, :, HD:HD + 1], 1.0)

    wq_sb = ap.tile([128, NDC, E2], BF16)
    nc.sync.dma_start(out=wq_sb,
                      in_=wq_d.ap().rearrange("(c p) m -> p c m", p=128))
    wk_sb = ap.tile([128, NDC, E2], BF16)
    nc.sync.dma_start(out=wk_sb,
                      in_=wk_d.ap().rearrange("(c p) m -> p c m", p=128))
    wv_sb = ap.tile([128, NDC, E2], BF16)
    nc.sync.dma_start(out=wv_sb,
                      in_=wv_d.ap().rearrange("(c p) m -> p c m", p=128))
    mk_sb = ap.tile([128, 896], BF16)
    nc.sync.dma_start(out=mk_sb, in_=mk_d.ap())

    for tt in range(NTOK // 512):
        for w_sb, dstT in ((wq_sb, qT), (wk_sb, kT)):
            acc = psM.tile([128, 512], F32, tag="big", bufs=4)
            for c in range(NDC):
                nc.tensor.matmul(acc, lhsT=w_sb[:, c, :],
                                 rhs=xnT[:, c, ts(tt, 512)],
                                 start=(c == 0), stop=(c == NDC - 1))
            nc.vector.tensor_mul(out=dstT[:, ts(tt, 512)], in0=acc,
                                 in1=nbc_all[:, tt, :])
    for gt in range(NTOK // 128):
        acc = psM.tile([128, E2], F32, tag="big", bufs=4)
        for c in range(NDC):
            nc.tensor.matmul(acc, lhsT=xnT[:, c, ts(gt, 128)],
                             rhs=wv_sb[:, c, :],
                             start=(c == 0), stop=(c == NDC - 1))
        nc.vector.tensor_scalar_mul(out=v0[:, gt, 0:HD], in0=acc[:, 0:HD],
                                    scalar1=scale_t[:, gt:gt + 1])
        nc.vector.tensor_scalar_mul(out=v1[:, gt, 0:HD], in0=acc[:, HD:E2],
                                    scalar1=scale_t[:, gt:gt + 1])

    if stop_after < 2:
        _finish(psm_cm, attn, dram, persist)
        return

    # ============ Phase 2: causal attention (2 heads, 2 batches) ===========
    for b in range(B):
        sums = ap.tile([8, 512], F32, tag=f"sums{b}")   # r = h*4 + tq
        recip_bf = ap.tile([8, 512], BF16, tag=f"recip{b}")
        for tq in range(T // 512):
            nblk = 4 * (tq + 1)
            q0 = b * T + tq * 512
            av0 = psM.tile([HD + 1, 512], F32, tag="av0")
            av1 = psM.tile([HD + 1, 512], F32, tag="av1")
            for blk in range(nblk):
                k0 = b * T + blk * 128
                s0 = psM.tile([128, 512], F32, tag="big", bufs=4)
                s1 = psM.tile([128, 512], F32, tag="big", bufs=4)
                nc.tensor.matmul(s0, lhsT=kT[0:HD, k0:k0 + 128],
                                 rhs=qT[0:HD, q0:q0 + 512])
                nc.tensor.matmul(s1, lhsT=kT[HD:E2, k0:k0 + 128],
                                 rhs=qT[HD:E2, q0:q0 + 512])
                pe0 = ap.tile([128, 512], BF16, tag="pe0", bufs=3)
                pe1 = ap.tile([128, 512], BF16, tag="pe1", bufs=3)
                nc.scalar.activation(out=pe0, in_=s0,
                                     func=AF.Exp, scale=HD ** -0.5)
                nc.scalar.activation(out=pe1, in_=s1,
                                     func=AF.Exp, scale=HD ** -0.5)
                if blk >= 4 * tq:               # diagonal block: mask
                    off = 384 - (blk - 4 * tq) * 128
                    nc.vector.tensor_mul(out=pe0, in0=pe0,
                                         in1=mk_sb[:, off:off + 512])
                    nc.vector.tensor_mul(out=pe1, in0=pe1,
                                         in1=mk_sb[:, off:off + 512])
                gt = (b * T) // 128 + blk
                nc.tensor.matmul(av0, lhsT=v0[:, gt, :], rhs=pe0,
                                 start=(blk == 0), stop=(blk == nblk - 1))
                nc.tensor.matmul(av1, lhsT=v1[:, gt, :], rhs=pe1,
                                 start=(blk == 0), stop=(blk == nblk - 1))
            # move unnormalized o + sums to SBUF
            for h, av in ((0, av0), (1, av1)):
                o_sb = ap.tile([HD + 1, 512], F32, tag=f"o{h}", bufs=4)
                nc.scalar.copy(out=o_sb, in_=av)
                r = h * 4 + tq
                # cross-partition move of the sum row via SBUF->SBUF DMA
                nc.gpsimd.dma_start(out=sums[r:r + 1, :],
                                    in_=o_sb[HD:HD + 1, :])
                _O_STASH[(b, tq, h)] = o_sb
        # after all tq of this batch: reciprocal + normalize + emit
        with nc.allow_low_precision(reason="softmax denom bcast in bf16"):
            nc.vector.reciprocal(out=recip_bf, in_=sums)
        for tq in range(T // 512):
            for h in range(HPC):
                r = h * 4 + tq
                bc = psM.tile([HD, 512], F32, tag="small", bufs=2)
                nc.tensor.matmul(bc, lhsT=sel8[:, r, :], rhs=recip_bf)
                o_sb = _O_STASH.pop((b, tq, h))
                chunk = ap.tile([HD, 512], BF16, tag="nrm", bufs=2)
                nc.vector.tensor_mul(out=chunk, in0=o_sb[0:HD, :], in1=bc)
                slot = b * 4 + tq
                nc.sync.dma_start(
                    out=a2a_in[slot, h * HD:(h + 1) * HD, :], in_=chunk)

    _finish(psm_cm, attn)

    if stop_after < 3:
        _finish(dram, persist)
        return

    # ========== Phases 3-6: A2A, proj, RMS2, GLU FFN + down-proj ==========
    if variant in ("nocc", "fastw"):
        nc.sync.dma_start(out=a2a_out[:], in_=a2a_in[:])
    else:
        nc.gpsimd.collective_compute(
            "AllToAll", ALU.bypass, replica_groups=[CORE_IDS],
            ins=[a2a_in[:].opt()], outs=[a2a_out[:].opt()])

    psf_cm = tc.tile_pool(name="psF", bufs=1, space="PSUM")
    psF = psf_cm.__enter__()
    ffn = tc.tile_pool(name="ffn", bufs=1)
    fp = ffn.__enter__()
    xn2T = fp.tile([128, NDC, TS], BF16)
    ident = fp.tile([128, 128], BF16)
    from concourse.masks import make_identity
    make_identity(nc, ident)
    gluT = fp.tile([128, 32, TS], BF16)   # gluT[p, i, t] = glu[t, i*128+p]

    p5_cm = tc.tile_pool(name="p5", bufs=2)
    p5 = p5_cm.__enter__()
    # prefetch FFN weights early: these DMAs are dependency-free and overlap
    # the AllToAll + projection + RMS2 window
    w1_tiles, w2_tiles = {}, {}
    def load_w1(qr):
        w1_t = p5.tile([128, NDC, 2048], BF16, tag="w1", name=f"w1q{qr}")
        lo = w1_d.ap()[:, qr * 1024:(qr + 1) * 1024]
        hi = w1_d.ap()[:, 4 * D + qr * 1024:4 * D + (qr + 1) * 1024]
        if variant == "fastw":
            nc.sync.dma_start(out=w1_t[:, 0:1, 0:1024],
                              in_=lo.rearrange("(c p) m -> p c m", p=128)[:, 0:1, :])
        else:
            nc.sync.dma_start(out=w1_t[:, :, 0:1024],
                              in_=lo.rearrange("(c p) m -> p c m", p=128))
            nc.sync.dma_start(out=w1_t[:, :, 1024:2048],
                              in_=hi.rearrange("(c p) m -> p c m", p=128))
        w1_tiles[qr] = w1_t
    def load_w2(qr):
        w2q = p5.tile([128, 8, D], BF16, tag="w2q", name=f"w2q{qr}")
        srcq = w2_d.ap()[qr * 1024:(qr + 1) * 1024, :]
        if variant == "fastw":
            nc.sync.dma_start(out=w2q[:, 0:1, :],
                              in_=srcq.rearrange("(i p) n -> p i n", p=128)[:, 0:1, :])
        else:
            nc.sync.dma_start(out=w2q,
                              in_=srcq.rearrange("(i p) n -> p i n", p=128))
        w2_tiles[qr] = w2q
    load_w1(0)
    load_w2(0)

    # ---- projection (reads A2A output) ----
    with tc.tile_pool(name="proj", bufs=1) as pj:
        oT_sb = pj.tile([128, NCORES, TS], BF16)
        nc.sync.dma_start(out=oT_sb,
                          in_=a2a_out[:].rearrange("s p f -> p s f"))
        wp_sb = pj.tile([128, NDC, D], BF16)
        nc.sync.dma_start(out=wp_sb,
                          in_=wp_d.ap().rearrange("(c p) n -> p c n", p=128))
        for tb in range(TS // 128):
            for dt in range(D // 512):
                acc = psF.tile([128, 512], F32, tag="xo", bufs=2)
                for c in range(NDC):
                    nc.tensor.matmul(acc, lhsT=oT_sb[:, c, ts(tb, 128)],
                                     rhs=wp_sb[:, c, ts(dt, 512)],
                                     start=(c == 0), stop=(c == NDC - 1))
                nc.vector.tensor_add(out=xsp_sb[:, tb, ts(dt, 512)],
                                     in0=acc, in1=xsp_sb[:, tb, ts(dt, 512)])

    if stop_after < 4:
        _finish(p5_cm, ffn, psf_cm, dram, persist)
        return

    # ---- second RMSNorm + on-chip PE transpose ----
    for tb in range(TS // 128):
        sq = p5.tile([128, D], BF16, tag="sq4")
        ssum = p5.tile([128, 1], F32, tag="ssum4")
        nc.scalar.activation(out=sq, in_=xsp_sb[:, tb, :], func=AF.Square,
                             accum_out=ssum)
        rs = p5.tile([128, 1], F32, tag="rs4")
        nc.scalar.activation(out=rs, in_=ssum, func=AF.Sqrt,
                             bias=eps_t, scale=1.0 / D)
        nc.vector.reciprocal(out=rs, in_=rs)
        xn2_t = p5.tile([128, D], BF16, tag="xn2t")
        nc.vector.tensor_scalar_mul(out=xn2_t, in0=xsp_sb[:, tb, :],
                                    scalar1=rs)
        for c in range(NDC):
            tp = psF.tile([128, 128], BF16, tag="xo", bufs=2)
            nc.tensor.transpose(tp, xn2_t[:, ts(c, 128)], ident)
            nc.vector.tensor_copy(out=xn2T[:, c, ts(tb, 128)], in_=tp)

    if stop_after < 5:
        _finish(p5_cm, ffn, psf_cm, dram, persist)
        return

    # ---- GLU FFN with interleaved down-projection ----
    for qr in range(4):
        if qr + 1 < 4:
            load_w1(qr + 1)
            load_w2(qr + 1)
        w1_t = w1_tiles.pop(qr)
        for i_loc in range(8):
            ia = qr * 8 + i_loc
            ha = psF.tile([128, TS], F32, tag="ha", bufs=2)
            for c in range(NDC):
                nc.tensor.matmul(ha, lhsT=w1_t[:, c, ts(i_loc, 128)],
                                 rhs=xn2T[:, c, :],
                                 start=(c == 0), stop=(c == NDC - 1))
            aT = p5.tile([128, TS], BF16, tag="aT")
            nc.vector.tensor_scalar_add(out=aT, in0=ha,
                                        scalar1=b1a_sb[:, ia:ia + 1])
            hb = psF.tile([128, TS], F32, tag="hb", bufs=2)
            for c in range(NDC):
                nc.tensor.matmul(hb,
                                 lhsT=w1_t[:, c, 1024 + i_loc * 128:
                                           1024 + (i_loc + 1) * 128],
                                 rhs=xn2T[:, c, :],
                                 start=(c == 0), stop=(c == NDC - 1))
            sg = p5.tile([128, TS], BF16, tag="sg")
            nc.scalar.activation(out=sg, in_=hb, func=AF.Sigmoid,
                                 bias=b1b_sb[:, ia:ia + 1])
            nc.vector.tensor_mul(out=gluT[:, ia, :], in0=aT, in1=sg)
        if qr % 2 == 1:
            half = qr // 2
            w2a = w2_tiles.pop(qr - 1)
            w2b = w2_tiles.pop(qr)
            for tb in range(TS // 128):
                for dt in range(D // 512):
                    yy = psF.tile([128, 512], F32, tag="yy", bufs=2)
                    for j in range(16):
                        i = half * 16 + j
                        w2t = w2a if j < 8 else w2b
                        nc.tensor.matmul(
                            yy, lhsT=gluT[:, i, ts(tb, 128)],
                            rhs=w2t[:, i % 8, ts(dt, 512)],
                            start=(j == 0), stop=(j == 15 and half == 1))
                    if half == 0:
                        nc.tensor.matmul(yy, lhsT=ones1,
                                         rhs=b2_sb[:, ts(dt, 512)],
                                         start=False, stop=True)
                        nc.vector.tensor_add(
                            out=xsp_sb[:, tb, ts(dt, 512)], in0=yy,
                            in1=xsp_sb[:, tb, ts(dt, 512)])
                    else:
                        out_t = p5.tile([128, 512], F32, tag="out", bufs=2)
                        nc.vector.tensor_add(
                            out=out_t, in0=yy,
                            in1=xsp_sb[:, tb, ts(dt, 512)])
                        nc.sync.dma_start(
                            out=out_d.ap()[ts(tb, 128), ts(dt, 512)],
                            in_=out_t)

    _finish(p5_cm, ffn, psf_cm, dram, persist)


def _prep_inputs(x, wq, wk, wv, w_proj, b_proj, w1, b1, w2, b2, g1, g2):
    bf16 = ml_dtypes.bfloat16
    xf = np.asarray(x, np.float32).reshape(NTOK, D)
    g1 = np.asarray(g1, np.float32)
    g2 = np.asarray(g2, np.float32)
    wqf = (np.asarray(wq, np.float32) * g1[None, :, None]).astype(bf16)
    wkf = (np.asarray(wk, np.float32) * g1[None, :, None]).astype(bf16)
    wvf = (np.asarray(wv, np.float32) * g1[None, :, None]).astype(bf16)
    w1f = np.ascontiguousarray(
        (np.asarray(w1, np.float32) * g2[:, None]).astype(bf16))
    w2b = np.ascontiguousarray(np.asarray(w2, np.float32).astype(bf16))
    wpb = np.ascontiguousarray(np.asarray(w_proj, np.float32).astype(bf16))
    b2r = np.ascontiguousarray(
        np.asarray(b2, np.float32).astype(bf16).reshape(1, D))
    b1f = np.ascontiguousarray(np.asarray(b1, np.float32))
    b1rb = np.ascontiguousarray(
        np.asarray(b1, np.float32)[:4 * D].astype(bf16))
    bp = np.asarray(b_proj, np.float32)
    xtr = np.ascontiguousarray(xf.T.astype(bf16))
    mk = np.ascontiguousarray(
        (np.arange(128)[:, None] <= (np.arange(896)[None, :] - 384))
        .astype(bf16))
    sel = np.ascontiguousarray(
        np.broadcast_to(np.eye(8, dtype=bf16)[:, :, None], (8, 8, 64)))
    osel = np.ascontiguousarray(
        np.broadcast_to(np.eye(8, dtype=bf16)[None, :, :], (128, 8, 8))
        .transpose(0, 2, 1))  # osel[p, r, m] = (m == r)
    bsel = np.ascontiguousarray(
        np.broadcast_to(np.eye(8, dtype=bf16)[:, :, None], (8, 8, 128)))

    in_maps = []
    for c in range(NCORES):
        h0, h1 = HPC * c, HPC * c + 1
        in_maps.append({
            "xt": xtr,
            "wq": np.ascontiguousarray(np.concatenate([wqf[h0], wqf[h1]], 1)),
            "wk": np.ascontiguousarray(np.concatenate([wkf[h0], wkf[h1]], 1)),
            "wv": np.ascontiguousarray(np.concatenate([wvf[h0], wvf[h1]], 1)),
            "wp": wpb,
            "w1": w1f,
            "b1": b1f,
            "b1r": b1rb,
            "w2": w2b,
            "b2": b2r,
            "xsp": np.ascontiguousarray(xf[TS * c:TS * (c + 1)] + bp[None, :]),
            "mk": mk,
            "sel": sel,
            "osel": osel,
            "bsel": bsel,
        })
    return in_maps


def kernel(**inputs):
    in_maps = _prep_inputs(**inputs)
    if "nc" not in _CACHE:
        _CACHE["nc"] = _build()
    res = run_bass_kernel_spmd(_CACHE["nc"], in_maps, CORE_IDS)
    out = np.concatenate([res.results[c]["out"] for c in range(NCORES)], 0)
    return out.reshape(B, T, D).astype(np.float32)


if __name__ == "__main__":
    import reference
    inputs = {k: np.asarray(v) for k, v in reference.setup_inputs().items()}
    got = kernel(**inputs)
    want = np.asarray(reference.reference(**inputs))
    err = np.abs(got - want)
    scale = np.abs(want).max()
    print("max abs err:", err.max(), "scale:", scale)
    print("rel err (max/scale):", err.max() / scale)
